# revision 1
# baseline (speedup 1.0000x reference)
"""Multi-head attention (B=1, S=4096, D=1024, H=16, Hd=64) on 8 Trainium2 cores.

Sharding: tensor-parallel over heads — 2 heads per core. Each core computes
q/k/v projections for its 2 heads (128 dims), flash-style attention without
max-subtraction (scores are ~N(0,1) after scaling so exp never overflows),
and a partial output projection with its 128 rows of wo. Host sums the 8
partial outputs and adds bo.

All matmuls run as float32r (full-rate fp32 PE mode, ~1.5e-4 rel err).

Layouts on device (per core):
  xT   [D, S]      streamed in blocks of [128 (d-chunk), 512 (s)]
  qT/kT[128, S]    partitions = head dims (h0: 0-63, h1: 64-127)
  v    [128, 2, 65] per k-chunk: partitions = seq rows, last col = ones
                   (so attn@v_aug also yields the softmax denominator)
  scores^T psum [128 (k rows), 3x512 (q)] -> exp on ACT (1536-wide),
                   double-buffered 3-bank staging; next-Q groups hoisted
  ctx^T psum [65, 512] per head, accumulated over 32 k-chunks; the two ctx
                   banks double as psum for q-proj/transpose/bcast/out-proj
  out   [S, D]     natural layout, normalized via K=1 broadcast-matmul + recip
"""

import os
import sys
import types

import numpy as np

S = 4096
D = 1024
H = 16
HD = 64
N_CORES = 8
HPC = H // N_CORES  # heads per core = 2
DC = D // 128       # d-chunks = 8
QB = 512            # q block
GK = 2              # k-chunks per exp staging group (2 kc x 2 heads = 2048 free)

_LAST_EXEC_NS = None


def _install_ntff_hook_shim():
    if "antenv.axon_hooks" in sys.modules:
        return
    try:
        import antenv
        from trn_agent_boot.trn_boot import _ntff_profile_via_ctypes

        hook = _ntff_profile_via_ctypes("/opt/axon/libaxon_pjrt.so")
    except Exception:
        return
    mod = types.ModuleType("antenv.axon_hooks")
    _state = {"hook": hook}
    mod.get_axon_ntff_profile_hook = lambda: _state["hook"]
    mod.set_axon_ntff_profile_hook = lambda h: _state.update(hook=h)
    sys.modules["antenv.axon_hooks"] = mod
    antenv.axon_hooks = mod


def _build(s=S):
    import concourse.bass as bass
    import concourse.mybir as mybir
    import concourse.tile as tile
    from concourse import bacc
    from concourse.masks import make_identity

    f32 = mybir.dt.float32
    f32r = mybir.dt.float32r
    Exp = mybir.ActivationFunctionType.Exp

    KC = s // 128     # k-chunks
    PB = 512          # projection block
    NP = s // PB      # projection blocks
    QB = 512          # attention q block (== PB)
    GS = 3            # (kc, h) slices per exp staging group

    nc = bacc.Bacc("TRN2", target_bir_lowering=False, debug=False,
                   num_devices=N_CORES)

    NPb = s // 512
    xT_d = nc.declare_dram_parameter("xT", [NPb, 128, D // 128, 512], f32,
                                     isOutput=False)
    wq_d = nc.declare_dram_parameter("wq", [128, D], f32, isOutput=False)
    wk_d = nc.declare_dram_parameter("wk", [128, D], f32, isOutput=False)
    wv_d = nc.declare_dram_parameter("wv", [128, D], f32, isOutput=False)
    bq_d = nc.declare_dram_parameter("bq", [128, 1], f32, isOutput=False)
    bk_d = nc.declare_dram_parameter("bk", [128, 1], f32, isOutput=False)
    bv_d = nc.declare_dram_parameter("bv", [128, 1], f32, isOutput=False)
    wo_d = nc.declare_dram_parameter("wo", [128, D], f32, isOutput=False)
    out_d = nc.declare_dram_parameter("out", [s, D], f32, isOutput=True)


    with tile.TileContext(nc) as tc:
        import contextlib
        with contextlib.ExitStack() as ctx:
            wpool = ctx.enter_context(tc.tile_pool(name="w", bufs=1))
            xpool = ctx.enter_context(tc.tile_pool(name="x", bufs=2))
            kpool = ctx.enter_context(tc.tile_pool(name="kt", bufs=1))
            qpool = ctx.enter_context(tc.tile_pool(name="qt", bufs=NP))
            vpool = ctx.enter_context(tc.tile_pool(name="v4", bufs=KC))
            vtpool = ctx.enter_context(tc.tile_pool(name="vt", bufs=2))
            epool = ctx.enter_context(tc.tile_pool(name="ex", bufs=4))
            epool2 = ctx.enter_context(tc.tile_pool(name="ex2", bufs=4))
            cpool = ctx.enter_context(tc.tile_pool(name="ctxs", bufs=2))
            spool = ctx.enter_context(tc.tile_pool(name="sums", bufs=2))
            rpool = ctx.enter_context(tc.tile_pool(name="recb", bufs=2))
            opool = ctx.enter_context(tc.tile_pool(name="outs", bufs=3))
            # PSUM: 2x3 (stage) + 1 (ctx0) + 1 (ctx1) = 8 banks; the two ctx
            # banks double as psum for transposes/q-proj/broadcast/out-proj
            # between accumulation epochs (same tags, sequential reuse).
            stg = ctx.enter_context(tc.tile_pool(name="stg", bufs=2, space="PSUM"))
            cp = ctx.enter_context(tc.tile_pool(name="cp", bufs=1, space="PSUM"))

            # ---- constants / weights ----
            wq_t = wpool.tile([128, D], f32r, tag="wq")
            wk_t = wpool.tile([128, D], f32r, tag="wk")
            wv_t = wpool.tile([128, D], f32r, tag="wv")
            wo0_t = wpool.tile([64, D], f32r, tag="wo0")
            wo1_t = wpool.tile([64, D], f32r, tag="wo1")
            bq_t = wpool.tile([128, 1], f32, tag="bq")
            bk_t = wpool.tile([128, 1], f32, tag="bk")
            bv_t = wpool.tile([128, 1], f32, tag="bv")
            ident = wpool.tile([128, 128], f32, tag="ident")
            ones_f = wpool.tile([65, 64], f32, tag="ones_f")
            ones_t = wpool.tile([65, 64], f32r, tag="ones")
            onecol = wpool.tile([128, 2, 1], f32, tag="onecol")

            nc.sync.dma_start(wq_t[:], wq_d[:].bitcast(f32r))
            nc.sync.dma_start(wk_t[:], wk_d[:].bitcast(f32r))
            nc.sync.dma_start(wv_t[:], wv_d[:].bitcast(f32r))
            nc.sync.dma_start(wo0_t[:], wo_d[0:64, :].bitcast(f32r))
            nc.sync.dma_start(wo1_t[:], wo_d[64:128, :].bitcast(f32r))
            nc.sync.dma_start(bq_t[:], bq_d[:])
            nc.sync.dma_start(bk_t[:], bk_d[:])
            nc.sync.dma_start(bv_t[:], bv_d[:])
            make_identity(nc, ident[:])
            nc.vector.memset(ones_f[:], 1.0)
            nc.vector.tensor_copy(ones_t[:], ones_f[:])
            nc.vector.memset(onecol[:], 1.0)

            kT = kpool.tile([128, s], f32r, tag="kT")
            q_tiles = []
            v_tiles = []

            def mm(out, lhsT, rhs, start, stop):
                return nc.tensor.matmul(out, lhsT, rhs, start=start, stop=stop)

            def proj_block(w_t, b, dst_ap, bias_t, psum_pool, psum_tag, xb):
                ps = psum_pool.tile([128, PB], f32, tag=psum_tag)
                for c in range(DC):
                    mm(ps[:], w_t[:, c * 128:(c + 1) * 128], xb[:, c, :],
                       start=(c == 0), stop=(c == DC - 1))
                nc.vector.tensor_scalar_add(dst_ap, ps[:], bias_t[:])

            # ---- projections: all kT first, then qb0 (so attention Q0 can
            # start early); v blocks + remaining q blocks trail into attention
            for b in range(NP):
                xb = xpool.tile([128, DC, PB], f32r, tag="xb")
                nc.sync.dma_start(xb[:], xT_d[b].bitcast(f32r))
                proj_block(wk_t, b, kT[:, b * PB:(b + 1) * PB], bk_t,
                           stg, "stage", xb)
                qb = qpool.tile([128, PB], f32r, tag="qT")
                proj_block(wq_t, b, qb[:], bq_t, cp, "ctx0", xb)
                q_tiles.append(qb)
                vt = vtpool.tile([128, PB], f32, tag="vt")
                proj_block(wv_t, b, vt[:], bv_t, stg, "stage", xb)
                for j in range(PB // 128):
                    kc = b * (PB // 128) + j
                    tp = cp.tile([128, 128], f32, tag="ctx1")
                    nc.tensor.transpose(tp[:], vt[:, j * 128:(j + 1) * 128],
                                        ident[:])
                    v4 = vpool.tile([128, 2, 65], f32r, tag="v4")
                    nc.vector.tensor_copy(v4[:, :, 64:65], onecol[:])
                    nc.vector.tensor_copy(
                        v4[:, :, 0:64],
                        tp[:].rearrange("p (h m) -> p h m", h=2))
                    v_tiles.append(v4)

            # flat (kc, h) slice list, staged in ragged groups of GS;
            # (kc,h0),(kc,h1) stay adjacent so the K=64 row-tiled pairs overlap
            slices = [(kc, h) for kc in range(KC) for h in range(2)]
            groups = [slices[i:i + GS] for i in range(0, len(slices), GS)]

            # ---- attention (q-proj interleaved), normalize, out-proj ----
            def emit_scores_exp(qb, gi):
                grp = groups[gi]
                ns = len(grp)
                st = stg.tile([128, GS, QB], f32, tag="stage")
                epl = epool2 if gi < 4 else epool
                ex = epl.tile([128, GS, QB], f32r, tag="ex")
                for slot, (kc, h) in enumerate(grp):
                    mm(st[:, slot, :],
                       kT[h * 64:(h + 1) * 64, kc * 128:(kc + 1) * 128],
                       qb[h * 64:(h + 1) * 64, :],
                       start=True, stop=True)
                nc.scalar.activation(
                    ex[:, 0:ns, :], st[:, 0:ns, :], Exp,
                    bias=0.0, scale=float(1.0 / np.sqrt(HD)))
                return ex

            NG = len(groups)
            HOIST = 6
            hoisted = None
            for b in range(NP):
                Q = b
                qb = q_tiles[b]

                ctxp0 = cp.tile([65, QB], f32, tag="ctx0")
                ctxp1 = cp.tile([65, QB], f32, tag="ctx1")

                for gi, grp in enumerate(groups):
                    if gi < HOIST and hoisted is not None:
                        ex = hoisted[gi]
                    else:
                        ex = emit_scores_exp(qb, gi)
                    for slot, (kc, h) in enumerate(grp):
                        ctxp = ctxp0 if h == 0 else ctxp1
                        mm(ctxp[:], v_tiles[kc][:, h, :], ex[:, slot, :],
                           start=(kc == 0), stop=(kc == KC - 1))

                # hoist next Q's first groups ahead of this Q's epilogue so
                # ACT keeps streaming while the normalize chain resolves
                if b + 1 < NP:
                    hoisted = [emit_scores_exp(q_tiles[b + 1], gi)
                               for gi in range(HOIST)]
                else:
                    hoisted = None

                # normalize
                cs0 = cpool.tile([64, QB], f32r, tag="cs0")
                cs1 = cpool.tile([64, QB], f32r, tag="cs1")
                sums = spool.tile([65, 2 * QB], f32r, tag="sums")
                nc.vector.tensor_copy(cs0[:], ctxp0[0:64, :])
                nc.vector.tensor_copy(cs1[:], ctxp1[0:64, :])
                nc.vector.tensor_copy(sums[64:65, 0:QB], ctxp0[64:65, :])
                nc.vector.tensor_copy(sums[64:65, QB:2 * QB], ctxp1[64:65, :])
                rb0 = cp.tile([64, QB], f32, tag="ctx0")
                rb1 = cp.tile([64, QB], f32, tag="ctx1")
                mm(rb0[:], ones_t[64:65, :], sums[64:65, 0:QB],
                   start=True, stop=True)
                mm(rb1[:], ones_t[64:65, :], sums[64:65, QB:2 * QB],
                   start=True, stop=True)
                rec = rpool.tile([64, 2, QB], f32, tag="rec")
                nc.vector.reciprocal_approx_fast(rec[:, 0, :], rb0[:])
                nc.vector.reciprocal_approx_fast(rec[:, 1, :], rb1[:])
                nc.vector.tensor_mul(cs0[:], cs0[:], rec[:, 0, :])
                nc.vector.tensor_mul(cs1[:], cs1[:], rec[:, 1, :])

                # out-proj: out[m-block, :] = cs0.T@wo0 + cs1.T@wo1
                for m in range(QB // 128):
                    for nh in range(D // 512):
                        op = cp.tile([128, 512], f32, tag="ctx%d" % (m % 2))
                        mm(op[:], cs0[:, m * 128:(m + 1) * 128],
                           wo0_t[:, nh * 512:(nh + 1) * 512],
                           start=True, stop=False)
                        mm(op[:], cs1[:, m * 128:(m + 1) * 128],
                           wo1_t[:, nh * 512:(nh + 1) * 512],
                           start=False, stop=True)
                        ob = opool.tile([128, 512], f32, tag="ob")
                        nc.vector.tensor_copy(ob[:], op[:])
                        nc.sync.dma_start(
                            out_d[Q * QB + m * 128:Q * QB + (m + 1) * 128,
                                  nh * 512:(nh + 1) * 512],
                            ob[:])

    nc.compile()
    return nc


def _shard_inputs(x, wq, bq, wk, bk, wv, bv, wo, bo, s):
    # [D, s] -> contiguous per-block layout [s//512, 128, D//128, 512]
    xT2 = np.asarray(x, np.float32).reshape(s, D).T
    xT = np.ascontiguousarray(
        xT2.reshape(D // 128, 128, s // 512, 512).transpose(2, 1, 0, 3))

    def lhsT_layout(w, c):
        blk = np.asarray(w, np.float32)[:, c * 128:(c + 1) * 128]
        return np.ascontiguousarray(
            blk.reshape(DC, 128, 128).transpose(1, 0, 2).reshape(128, D))

    in_maps = []
    for c in range(N_CORES):
        in_maps.append({
            "xT": xT,
            "wq": lhsT_layout(wq, c),
            "wk": lhsT_layout(wk, c),
            "wv": lhsT_layout(wv, c),
            "bq": np.ascontiguousarray(
                np.asarray(bq, np.float32)[c * 128:(c + 1) * 128, None]),
            "bk": np.ascontiguousarray(
                np.asarray(bk, np.float32)[c * 128:(c + 1) * 128, None]),
            "bv": np.ascontiguousarray(
                np.asarray(bv, np.float32)[c * 128:(c + 1) * 128, None]),
            "wo": np.ascontiguousarray(
                np.asarray(wo, np.float32)[c * 128:(c + 1) * 128, :]),
        })
    return in_maps


def run(x, wq, bq, wk, bk, wv, bv, wo, bo, trace=False, s=S):
    global _LAST_EXEC_NS
    from concourse.bass_utils import run_bass_kernel_spmd

    if trace:
        _install_ntff_hook_shim()
    nc = _build(s)
    in_maps = _shard_inputs(x, wq, bq, wk, bk, wv, bv, wo, bo, s)
    res = run_bass_kernel_spmd(nc, in_maps, core_ids=list(range(N_CORES)),
                               trace=trace)
    _LAST_EXEC_NS = res.exec_time_ns
    out = res.results[0]["out"].astype(np.float64)
    for c in range(1, N_CORES):
        out += res.results[c]["out"]
    out += np.asarray(bo, np.float64)
    return out.astype(np.float32).reshape(1, s, D)


def kernel(x, wq, bq, wk, bk, wv, bv, wo, bo):
    trace = bool(os.environ.get("BASS_MHA_TRACE"))
    return run(x, wq, bq, wk, bk, wv, bv, wo, bo, trace=trace)



# revision 2
# speedup vs baseline: 1.2205x; 1.2205x over previous
"""Multi-head attention (B=1, S=4096, D=1024, H=16, Hd=64) on 8 Trainium2 cores.

Sharding: tensor-parallel over heads - 2 heads per core. Each core computes
q/k/v projections for its 2 heads (128 dims), flash-style attention without
max-subtraction (scores are ~N(0,1) after scaling so exp never overflows),
and a partial output projection with its 128 rows of wo. Host sums the 8
partial outputs and adds bo.

The exp stream on the scalar (ACT) engine is the roofline: 2 heads x 4096^2
= 33.5M exps per core at 1 elem/cycle/lane @1.2GHz ~= 250us including
per-instruction overhead. Everything else is arranged to hide under it:

  - all matmul operands are fp16 (2-byte moving operand streams 2 cols/cycle;
    FWL halves weight loads; x DMA halves to 8.4MB). PSUM stays fp32.
  - v is produced directly in attention layout [k-rows, head, 65] by using
    the x^T chunk as the matmul stationary and an augmented wv (64 cols h0 |
    zero | 64 cols h1 | zero) as moving; the zero columns get +1.0 from a
    host-prepared bias tile, fusing the softmax-denominator ones trick with
    the v bias add. No PE transposes needed.
  - the score matmuls are emitted one group ahead of the ctx matmuls
    (software pipeline) so the ACT engine always has the next staged group
    ready: PE order [scores g+1][ctx g] instead of [scores g][ctx g].
  - the first HOIST groups of q-block b+1 are emitted before b's epilogue
    so ACT keeps streaming while the normalize/out-proj chain resolves.

Layouts on device (per core):
  xT   [8, 128, 512] fp16 per block: partitions = d-chunk dims
  qT/kT[128, S] fp16   partitions = head dims (h0: 0-63, h1: 64-127)
  v4   [128, 2, 65] fp16 per k-chunk: partitions = seq rows, col 64 = ones
  scores psum [128 (k rows), 3x512 (q)] fp32 -> exp on ACT -> ex fp16
  ctx^T psum [65, 512] fp32 per head, accumulated over 32 k-chunks
  out  [S, D] fp16 partials, summed + bo on host
"""

import os
import sys
import types

import numpy as np

S = 4096
D = 1024
H = 16
HD = 64
N_CORES = 8
HPC = H // N_CORES  # heads per core = 2
DC = D // 128       # d-chunks = 8
QB = 512            # q block

_LAST_EXEC_NS = None


def _install_ntff_hook_shim():
    if "antenv.axon_hooks" in sys.modules:
        return
    try:
        import antenv
        from trn_agent_boot.trn_boot import _ntff_profile_via_ctypes

        hook = _ntff_profile_via_ctypes("/opt/axon/libaxon_pjrt.so")
    except Exception:
        return
    mod = types.ModuleType("antenv.axon_hooks")
    _state = {"hook": hook}
    mod.get_axon_ntff_profile_hook = lambda: _state["hook"]
    mod.set_axon_ntff_profile_hook = lambda h: _state.update(hook=h)
    sys.modules["antenv.axon_hooks"] = mod
    antenv.axon_hooks = mod


def _build(s=S):
    import concourse.bass as bass
    import concourse.mybir as mybir
    import concourse.tile as tile
    from concourse import bacc

    f32 = mybir.dt.float32
    f16 = mybir.dt.float16
    Exp = mybir.ActivationFunctionType.Exp

    KC = s // 128     # k-chunks
    PB = 512          # projection block
    NP = s // PB      # projection blocks
    QB = 512          # attention q block (== PB)
    GS = 3            # (kc, h) slices per exp staging group
    HOIST = 8         # next-q groups emitted before current epilogue

    nc = bacc.Bacc("TRN2", target_bir_lowering=False, debug=False,
                   num_devices=N_CORES)

    xT_d = nc.declare_dram_parameter("xT", [NP, 128, DC, 512], f16,
                                     isOutput=False)
    wq_d = nc.declare_dram_parameter("wq", [128, D], f16, isOutput=False)
    wk_d = nc.declare_dram_parameter("wk", [128, D], f16, isOutput=False)
    wv_d = nc.declare_dram_parameter("wv", [128, DC, 130], f16, isOutput=False)
    bq_d = nc.declare_dram_parameter("bq", [128, 1], f32, isOutput=False)
    bk_d = nc.declare_dram_parameter("bk", [128, 1], f32, isOutput=False)
    bvb_d = nc.declare_dram_parameter("bvb", [128, 2, 65], f16, isOutput=False)
    wo_d = nc.declare_dram_parameter("wo", [128, D], f16, isOutput=False)
    out_d = nc.declare_dram_parameter("out", [s, D], f16, isOutput=True)

    with tile.TileContext(nc) as tc:
        import contextlib
        with contextlib.ExitStack() as ctx:
            wpool = ctx.enter_context(tc.tile_pool(name="w", bufs=1))
            xpool = ctx.enter_context(tc.tile_pool(name="x", bufs=NP))
            kpool = ctx.enter_context(tc.tile_pool(name="kt", bufs=1))
            qpool = ctx.enter_context(tc.tile_pool(name="qt", bufs=NP))
            vpool = ctx.enter_context(tc.tile_pool(name="v4", bufs=KC))
            epool = ctx.enter_context(tc.tile_pool(name="ex", bufs=4))
            epool2 = ctx.enter_context(tc.tile_pool(name="ex2", bufs=HOIST))
            cpool = ctx.enter_context(tc.tile_pool(name="ctxs", bufs=2))
            spool = ctx.enter_context(tc.tile_pool(name="sums", bufs=2))
            rpool = ctx.enter_context(tc.tile_pool(name="recb", bufs=2))
            opool = ctx.enter_context(tc.tile_pool(name="outs", bufs=3))
            # PSUM: 2x3 (stage) + 1 (ctx0) + 1 (ctx1) = 8 banks. The two ctx
            # banks double as psum for q-proj/bcast/out-proj between
            # accumulation epochs; k/v-proj psum rotates through the stage
            # bufs (sequential reuse, tag-ordered).
            stg = ctx.enter_context(tc.tile_pool(name="stg", bufs=2, space="PSUM"))
            cp = ctx.enter_context(tc.tile_pool(name="cp", bufs=1, space="PSUM"))

            # ---- constants / weights ----
            wq_t = wpool.tile([128, D], f16, tag="wq")
            wk_t = wpool.tile([128, D], f16, tag="wk")
            wv_t = wpool.tile([128, DC, 130], f16, tag="wv")
            wo0_t = wpool.tile([64, D], f16, tag="wo0")
            wo1_t = wpool.tile([64, D], f16, tag="wo1")
            bq_t = wpool.tile([128, 1], f32, tag="bq")
            bk_t = wpool.tile([128, 1], f32, tag="bk")
            bvb_t = wpool.tile([128, 2, 65], f16, tag="bvb")
            ones_t = wpool.tile([65, 64], f16, tag="ones")

            nc.sync.dma_start(wq_t[:], wq_d[:])
            nc.sync.dma_start(wk_t[:], wk_d[:])
            nc.sync.dma_start(wv_t[:], wv_d[:])
            nc.sync.dma_start(wo0_t[:], wo_d[0:64, :])
            nc.sync.dma_start(wo1_t[:], wo_d[64:128, :])
            nc.sync.dma_start(bq_t[:], bq_d[:])
            nc.sync.dma_start(bk_t[:], bk_d[:])
            nc.sync.dma_start(bvb_t[:], bvb_d[:])
            nc.vector.memset(ones_t[:], 1.0)

            kT = kpool.tile([128, s], f16, tag="kT")
            q_tiles = []
            v_tiles = []

            def mm(out, lhsT, rhs, start, stop):
                return nc.tensor.matmul(out, lhsT, rhs, start=start, stop=stop)

            # ---- x: all 8 blocks DMA'd up front (free-running stream) ----
            x_tiles = []
            for b in range(NP):
                xb = xpool.tile([128, DC, PB], f16, tag="xb")
                nc.sync.dma_start(xb[:], xT_d[b])
                x_tiles.append(xb)

            # ---- projections ----
            def proj_block(w_t, dst_ap, bias_t, psum_pool, psum_tag, xb):
                ps = psum_pool.tile([128, PB], f32, tag=psum_tag)
                for c in range(DC):
                    mm(ps[:], w_t[:, c * 128:(c + 1) * 128], xb[:, c, :],
                       start=(c == 0), stop=(c == DC - 1))
                nc.vector.tensor_scalar_add(dst_ap, ps[:], bias_t[:])

            for b in range(NP):
                xb = x_tiles[b]
                proj_block(wk_t, kT[:, b * PB:(b + 1) * PB], bk_t,
                           stg, "stage", xb)
                qb = qpool.tile([128, PB], f16, tag="qT")
                proj_block(wq_t, qb[:], bq_t, cp, "ctx0", xb)
                q_tiles.append(qb)
                # v directly in attention layout: xb chunk stationary,
                # augmented wv moving; +bvb adds bias and the ones column.
                for j in range(PB // 128):
                    vps = stg.tile([128, 130], f32, tag="stage")
                    for c in range(DC):
                        mm(vps[:], xb[:, c, j * 128:(j + 1) * 128],
                           wv_t[:, c, :], start=(c == 0), stop=(c == DC - 1))
                    v4 = vpool.tile([128, 2, 65], f16, tag="v4")
                    nc.vector.tensor_add(
                        v4[:], vps[:].rearrange("p (h m) -> p h m", h=2),
                        bvb_t[:])
                    v_tiles.append(v4)

            # flat (kc, h) slice list, staged in ragged groups of GS;
            # (kc,h0),(kc,h1) stay adjacent so the K=64 row-tiled pairs overlap
            slices = [(kc, h) for kc in range(KC) for h in range(2)]
            groups = [slices[i:i + GS] for i in range(0, len(slices), GS)]
            NG = len(groups)

            # ---- attention: scores+exp pipelined one group ahead of ctx ----
            def emit_scores_exp(qb, gi):
                grp = groups[gi]
                ns = len(grp)
                st = stg.tile([128, GS, QB], f32, tag="stage")
                epl = epool2 if gi < HOIST else epool
                ex = epl.tile([128, GS, QB], f16, tag="ex")
                for slot, (kc, h) in enumerate(grp):
                    mm(st[:, slot, :],
                       kT[h * 64:(h + 1) * 64, kc * 128:(kc + 1) * 128],
                       qb[h * 64:(h + 1) * 64, :],
                       start=True, stop=True)
                nc.scalar.activation(
                    ex[:, 0:ns, :], st[:, 0:ns, :], Exp,
                    bias=0.0, scale=float(1.0 / np.sqrt(HD)))
                return ex

            hoisted = None
            for b in range(NP):
                Q = b
                qb = q_tiles[b]

                ctxp0 = cp.tile([65, QB], f32, tag="ctx0")
                ctxp1 = cp.tile([65, QB], f32, tag="ctx1")

                ex_tiles = {}
                if hoisted is not None:
                    for gi in range(HOIST):
                        ex_tiles[gi] = hoisted[gi]
                pend = [gi for gi in range(NG) if gi not in ex_tiles]
                # prefill the scores pipeline two groups deep
                for _ in range(min(2, len(pend))):
                    gi = pend.pop(0)
                    ex_tiles[gi] = emit_scores_exp(qb, gi)

                for gi, grp in enumerate(groups):
                    ex = ex_tiles.pop(gi)
                    for slot, (kc, h) in enumerate(grp):
                        ctxp = ctxp0 if h == 0 else ctxp1
                        mm(ctxp[:], v_tiles[kc][:, h, :], ex[:, slot, :],
                           start=(kc == 0), stop=(kc == KC - 1))
                    if pend:
                        ngi = pend.pop(0)
                        ex_tiles[ngi] = emit_scores_exp(qb, ngi)

                # hoist next Q's first groups ahead of this Q's epilogue so
                # ACT keeps streaming while the normalize chain resolves
                if b + 1 < NP:
                    hoisted = [emit_scores_exp(q_tiles[b + 1], gi)
                               for gi in range(HOIST)]
                else:
                    hoisted = None

                # normalize: ctx rows / denominator row via recip + bcast-mm
                cs0 = cpool.tile([64, QB], f16, tag="cs0")
                cs1 = cpool.tile([64, QB], f16, tag="cs1")
                sums = spool.tile([65, 2 * QB], f16, tag="sums")
                nc.vector.tensor_copy(cs0[:], ctxp0[0:64, :])
                nc.vector.tensor_copy(cs1[:], ctxp1[0:64, :])
                nc.vector.tensor_copy(sums[64:65, 0:QB], ctxp0[64:65, :])
                nc.vector.tensor_copy(sums[64:65, QB:2 * QB], ctxp1[64:65, :])
                rb0 = cp.tile([64, QB], f32, tag="ctx0")
                rb1 = cp.tile([64, QB], f32, tag="ctx1")
                mm(rb0[:], ones_t[64:65, :], sums[64:65, 0:QB],
                   start=True, stop=True)
                mm(rb1[:], ones_t[64:65, :], sums[64:65, QB:2 * QB],
                   start=True, stop=True)
                rec = rpool.tile([64, 2, QB], f32, tag="rec")
                nc.vector.reciprocal_approx_fast(rec[:, 0, :], rb0[:])
                nc.vector.reciprocal_approx_fast(rec[:, 1, :], rb1[:])
                nc.vector.tensor_mul(cs0[:], cs0[:], rec[:, 0, :])
                nc.vector.tensor_mul(cs1[:], cs1[:], rec[:, 1, :])

                # out-proj: out[m-block, :] = cs0.T@wo0 + cs1.T@wo1
                for m in range(QB // 128):
                    for nh in range(D // 512):
                        op = cp.tile([128, 512], f32, tag="ctx%d" % (m % 2))
                        mm(op[:], cs0[:, m * 128:(m + 1) * 128],
                           wo0_t[:, nh * 512:(nh + 1) * 512],
                           start=True, stop=False)
                        mm(op[:], cs1[:, m * 128:(m + 1) * 128],
                           wo1_t[:, nh * 512:(nh + 1) * 512],
                           start=False, stop=True)
                        ob = opool.tile([128, 512], f16, tag="ob")
                        nc.vector.tensor_copy(ob[:], op[:])
                        nc.sync.dma_start(
                            out_d[Q * QB + m * 128:Q * QB + (m + 1) * 128,
                                  nh * 512:(nh + 1) * 512],
                            ob[:])

    nc.compile()
    return nc


def _shard_inputs(x, wq, bq, wk, bk, wv, bv, wo, bo, s):
    # [D, s] -> contiguous per-block layout [s//512, 128, D//128, 512], fp16
    xT2 = np.asarray(x, np.float32).reshape(s, D).T
    xT = np.ascontiguousarray(
        xT2.reshape(D // 128, 128, s // 512, 512).transpose(2, 1, 0, 3)
    ).astype(np.float16)

    def lhsT_layout(w, c):
        blk = np.asarray(w, np.float32)[:, c * 128:(c + 1) * 128]
        return np.ascontiguousarray(
            blk.reshape(DC, 128, 128).transpose(1, 0, 2).reshape(128, D)
        ).astype(np.float16)

    def wv_aug_layout(w, c):
        # [128, DC, 130]: per d-chunk, [h0 cols | 0 | h1 cols | 0]
        blk = np.asarray(w, np.float32)[:, c * 128:(c + 1) * 128]  # [D, 128]
        aug = np.zeros((DC, 128, 130), np.float32)
        aug[:, :, 0:64] = blk[:, 0:64].reshape(DC, 128, 64)
        aug[:, :, 65:129] = blk[:, 64:128].reshape(DC, 128, 64)
        return np.ascontiguousarray(aug.transpose(1, 0, 2)).astype(np.float16)

    def bvb_layout(bv, c):
        # [128, 2, 65]: v bias broadcast over k-rows + ones column
        bvc = np.asarray(bv, np.float32)[c * 128:(c + 1) * 128]
        t = np.empty((2, 65), np.float32)
        t[0, 0:64] = bvc[0:64]
        t[1, 0:64] = bvc[64:128]
        t[:, 64] = 1.0
        return np.ascontiguousarray(
            np.broadcast_to(t, (128, 2, 65))).astype(np.float16)

    in_maps = []
    for c in range(N_CORES):
        in_maps.append({
            "xT": xT,
            "wq": lhsT_layout(wq, c),
            "wk": lhsT_layout(wk, c),
            "wv": wv_aug_layout(wv, c),
            "bq": np.ascontiguousarray(
                np.asarray(bq, np.float32)[c * 128:(c + 1) * 128, None]),
            "bk": np.ascontiguousarray(
                np.asarray(bk, np.float32)[c * 128:(c + 1) * 128, None]),
            "bvb": bvb_layout(bv, c),
            "wo": np.ascontiguousarray(
                np.asarray(wo, np.float32)[c * 128:(c + 1) * 128, :]
            ).astype(np.float16),
        })
    return in_maps


def run(x, wq, bq, wk, bk, wv, bv, wo, bo, trace=False, s=S):
    global _LAST_EXEC_NS
    from concourse.bass_utils import run_bass_kernel_spmd

    if trace:
        _install_ntff_hook_shim()
    nc = _build(s)
    in_maps = _shard_inputs(x, wq, bq, wk, bk, wv, bv, wo, bo, s)
    res = run_bass_kernel_spmd(nc, in_maps, core_ids=list(range(N_CORES)),
                               trace=trace)
    _LAST_EXEC_NS = res.exec_time_ns
    out = res.results[0]["out"].astype(np.float64)
    for c in range(1, N_CORES):
        out += res.results[c]["out"]
    out += np.asarray(bo, np.float64)
    return out.astype(np.float32).reshape(1, s, D)


def kernel(x, wq, bq, wk, bk, wv, bv, wo, bo):
    trace = bool(os.environ.get("BASS_MHA_TRACE"))
    return run(x, wq, bq, wk, bk, wv, bv, wo, bo, trace=trace)


# revision 3
# speedup vs baseline: 1.2210x; 1.0003x over previous
"""Multi-head attention (B=1, S=4096, D=1024, H=16, Hd=64) on 8 Trainium2 cores.

Sharding: tensor-parallel over heads - 2 heads per core. Each core computes
q/k/v projections for its 2 heads (128 dims), flash-style attention without
max-subtraction (scores are ~N(0,1) after scaling so exp never overflows),
and a partial output projection with its 128 rows of wo. Host sums the 8
partial outputs and adds bo.

The exp stream on the scalar (ACT) engine is the roofline: 2 heads x 4096^2
= 33.5M exps per core at 1 elem/cycle/lane @1.2GHz ~= 250us including
per-instruction overhead. Everything else is arranged to hide under it:

  - all matmul operands are fp16 (2-byte moving operand streams 2 cols/cycle;
    FWL halves weight loads; x DMA halves to 8.4MB). PSUM stays fp32.
  - v is produced directly in attention layout [k-rows, head, 65] by using
    the x^T chunk as the matmul stationary and an augmented wv (64 cols h0 |
    zero | 64 cols h1 | zero) as moving; the zero columns get +1.0 from a
    host-prepared bias tile, fusing the softmax-denominator ones trick with
    the v bias add. No PE transposes needed.
  - the score matmuls are emitted one group ahead of the ctx matmuls
    (software pipeline) so the ACT engine always has the next staged group
    ready: PE order [scores g+1][ctx g] instead of [scores g][ctx g].
  - the first HOIST groups of q-block b+1 are emitted before b's epilogue
    so ACT keeps streaming while the normalize/out-proj chain resolves.

Layouts on device (per core):
  xT   [8, 128, 512] fp16 per block: partitions = d-chunk dims
  qT/kT[128, S] fp16   partitions = head dims (h0: 0-63, h1: 64-127)
  v4   [128, 2, 65] fp16 per k-chunk: partitions = seq rows, col 64 = ones
  scores psum [128 (k rows), 3x512 (q)] fp32 -> exp on ACT -> ex fp16
  ctx^T psum [65, 512] fp32 per head, accumulated over 32 k-chunks
  out  [S, D] fp16 partials, summed + bo on host
"""

import os
import sys
import types

import numpy as np

S = 4096
D = 1024
H = 16
HD = 64
N_CORES = 8
HPC = H // N_CORES  # heads per core = 2
DC = D // 128       # d-chunks = 8
QB = 512            # q block
DT16 = os.environ.get('BASS_MHA_DT16', 'bf16')

_LAST_EXEC_NS = None


def _install_ntff_hook_shim():
    if "antenv.axon_hooks" in sys.modules:
        return
    try:
        import antenv
        from trn_agent_boot.trn_boot import _ntff_profile_via_ctypes

        hook = _ntff_profile_via_ctypes("/opt/axon/libaxon_pjrt.so")
    except Exception:
        return
    mod = types.ModuleType("antenv.axon_hooks")
    _state = {"hook": hook}
    mod.get_axon_ntff_profile_hook = lambda: _state["hook"]
    mod.set_axon_ntff_profile_hook = lambda h: _state.update(hook=h)
    sys.modules["antenv.axon_hooks"] = mod
    antenv.axon_hooks = mod


def _build(s=S):
    import concourse.bass as bass
    import concourse.mybir as mybir
    import concourse.tile as tile
    from concourse import bacc

    f32 = mybir.dt.float32
    f16 = (mybir.dt.bfloat16 if DT16 == 'bf16'
           else mybir.dt.float16)
    Exp = mybir.ActivationFunctionType.Exp

    KC = s // 128     # k-chunks
    PB = 512          # projection block
    NP = s // PB      # projection blocks
    QB = 512          # attention q block (== PB)
    GS = 3            # (kc, h) slices per exp staging group
    HOIST = 8         # next-q groups emitted before current epilogue

    nc = bacc.Bacc("TRN2", target_bir_lowering=False, debug=False,
                   num_devices=N_CORES)

    xT_d = nc.declare_dram_parameter("xT", [NP, 128, DC, 512], f16,
                                     isOutput=False)
    wq_d = nc.declare_dram_parameter("wq", [128, D], f16, isOutput=False)
    wk_d = nc.declare_dram_parameter("wk", [128, D], f16, isOutput=False)
    wv_d = nc.declare_dram_parameter("wv", [128, DC, 130], f16, isOutput=False)
    bq_d = nc.declare_dram_parameter("bq", [128, 1], f32, isOutput=False)
    bk_d = nc.declare_dram_parameter("bk", [128, 1], f32, isOutput=False)
    bvb_d = nc.declare_dram_parameter("bvb", [128, 2, 65], f16, isOutput=False)
    wo_d = nc.declare_dram_parameter("wo", [128, D], f16, isOutput=False)
    out_d = nc.declare_dram_parameter("out", [s, D], f16, isOutput=True)

    with tile.TileContext(nc) as tc:
        import contextlib
        with contextlib.ExitStack() as ctx:
            wpool = ctx.enter_context(tc.tile_pool(name="w", bufs=1))
            xpool = ctx.enter_context(tc.tile_pool(name="x", bufs=NP))
            kpool = ctx.enter_context(tc.tile_pool(name="kt", bufs=1))
            qpool = ctx.enter_context(tc.tile_pool(name="qt", bufs=NP))
            vpool = ctx.enter_context(tc.tile_pool(name="v4", bufs=KC))
            epool = ctx.enter_context(tc.tile_pool(name="ex", bufs=4))
            epool2 = ctx.enter_context(tc.tile_pool(name="ex2", bufs=HOIST))
            cpool = ctx.enter_context(tc.tile_pool(name="ctxs", bufs=2))
            spool = ctx.enter_context(tc.tile_pool(name="sums", bufs=2))
            rpool = ctx.enter_context(tc.tile_pool(name="recb", bufs=2))
            opool = ctx.enter_context(tc.tile_pool(name="outs", bufs=3))
            # PSUM: 2x3 (stage) + 1 (ctx0) + 1 (ctx1) = 8 banks. The two ctx
            # banks double as psum for q-proj/bcast/out-proj between
            # accumulation epochs; k/v-proj psum rotates through the stage
            # bufs (sequential reuse, tag-ordered).
            stg = ctx.enter_context(tc.tile_pool(name="stg", bufs=2, space="PSUM"))
            cp = ctx.enter_context(tc.tile_pool(name="cp", bufs=1, space="PSUM"))

            # ---- constants / weights ----
            wq_t = wpool.tile([128, D], f16, tag="wq")
            wk_t = wpool.tile([128, D], f16, tag="wk")
            wv_t = wpool.tile([128, DC, 130], f16, tag="wv")
            wo0_t = wpool.tile([64, D], f16, tag="wo0")
            wo1_t = wpool.tile([64, D], f16, tag="wo1")
            bq_t = wpool.tile([128, 1], f32, tag="bq")
            bk_t = wpool.tile([128, 1], f32, tag="bk")
            bvb_t = wpool.tile([128, 2, 65], f16, tag="bvb")
            ones_t = wpool.tile([65, 64], f16, tag="ones")

            nc.sync.dma_start(wq_t[:], wq_d[:])
            nc.sync.dma_start(wk_t[:], wk_d[:])
            nc.sync.dma_start(wv_t[:], wv_d[:])
            nc.sync.dma_start(wo0_t[:], wo_d[0:64, :])
            nc.sync.dma_start(wo1_t[:], wo_d[64:128, :])
            nc.sync.dma_start(bq_t[:], bq_d[:])
            nc.sync.dma_start(bk_t[:], bk_d[:])
            nc.sync.dma_start(bvb_t[:], bvb_d[:])
            nc.vector.memset(ones_t[:], 1.0)

            kT = kpool.tile([128, s], f16, tag="kT")
            q_tiles = []
            v_tiles = []

            def mm(out, lhsT, rhs, start, stop):
                return nc.tensor.matmul(out, lhsT, rhs, start=start, stop=stop)

            # ---- x: all 8 blocks DMA'd up front (free-running stream) ----
            x_tiles = []
            for b in range(NP):
                xb = xpool.tile([128, DC, PB], f16, tag="xb")
                nc.sync.dma_start(xb[:], xT_d[b])
                x_tiles.append(xb)

            # ---- projections ----
            def proj_block(w_t, dst_ap, bias_t, psum_pool, psum_tag, xb):
                ps = psum_pool.tile([128, PB], f32, tag=psum_tag)
                for c in range(DC):
                    mm(ps[:], w_t[:, c * 128:(c + 1) * 128], xb[:, c, :],
                       start=(c == 0), stop=(c == DC - 1))
                nc.vector.tensor_scalar_add(dst_ap, ps[:], bias_t[:])

            for b in range(NP):
                xb = x_tiles[b]
                proj_block(wk_t, kT[:, b * PB:(b + 1) * PB], bk_t,
                           stg, "stage", xb)
                qb = qpool.tile([128, PB], f16, tag="qT")
                proj_block(wq_t, qb[:], bq_t, cp, "ctx0", xb)
                q_tiles.append(qb)
                # v directly in attention layout: xb chunk stationary,
                # augmented wv moving; +bvb adds bias and the ones column.
                for j in range(PB // 128):
                    vps = stg.tile([128, 130], f32, tag="stage")
                    for c in range(DC):
                        mm(vps[:], xb[:, c, j * 128:(j + 1) * 128],
                           wv_t[:, c, :], start=(c == 0), stop=(c == DC - 1))
                    v4 = vpool.tile([128, 2, 65], f16, tag="v4")
                    nc.vector.tensor_add(
                        v4[:], vps[:].rearrange("p (h m) -> p h m", h=2),
                        bvb_t[:])
                    v_tiles.append(v4)

            # flat (kc, h) slice list, staged in ragged groups of GS;
            # (kc,h0),(kc,h1) stay adjacent so the K=64 row-tiled pairs overlap
            slices = [(kc, h) for kc in range(KC) for h in range(2)]
            groups = [slices[i:i + GS] for i in range(0, len(slices), GS)]
            NG = len(groups)

            # ---- attention: scores+exp pipelined one group ahead of ctx ----
            def emit_scores_exp(qb, gi):
                grp = groups[gi]
                ns = len(grp)
                st = stg.tile([128, GS, QB], f32, tag="stage")
                epl = epool2 if gi < HOIST else epool
                ex = epl.tile([128, GS, QB], f16, tag="ex")
                for slot, (kc, h) in enumerate(grp):
                    mm(st[:, slot, :],
                       kT[h * 64:(h + 1) * 64, kc * 128:(kc + 1) * 128],
                       qb[h * 64:(h + 1) * 64, :],
                       start=True, stop=True)
                nc.scalar.activation(
                    ex[:, 0:ns, :], st[:, 0:ns, :], Exp,
                    bias=0.0, scale=float(1.0 / np.sqrt(HD)))
                return ex

            hoisted = None
            for b in range(NP):
                Q = b
                qb = q_tiles[b]

                ctxp0 = cp.tile([65, QB], f32, tag="ctx0")
                ctxp1 = cp.tile([65, QB], f32, tag="ctx1")

                ex_tiles = {}
                if hoisted is not None:
                    for gi in range(HOIST):
                        ex_tiles[gi] = hoisted[gi]
                pend = [gi for gi in range(NG) if gi not in ex_tiles]
                # prefill the scores pipeline two groups deep
                for _ in range(min(2, len(pend))):
                    gi = pend.pop(0)
                    ex_tiles[gi] = emit_scores_exp(qb, gi)

                for gi, grp in enumerate(groups):
                    ex = ex_tiles.pop(gi)
                    for slot, (kc, h) in enumerate(grp):
                        ctxp = ctxp0 if h == 0 else ctxp1
                        mm(ctxp[:], v_tiles[kc][:, h, :], ex[:, slot, :],
                           start=(kc == 0), stop=(kc == KC - 1))
                    if pend:
                        ngi = pend.pop(0)
                        ex_tiles[ngi] = emit_scores_exp(qb, ngi)

                # hoist next Q's first groups ahead of this Q's epilogue so
                # ACT keeps streaming while the normalize chain resolves
                if b + 1 < NP:
                    hoisted = [emit_scores_exp(q_tiles[b + 1], gi)
                               for gi in range(HOIST)]
                else:
                    hoisted = None

                # normalize: ctx rows / denominator row via recip + bcast-mm
                cs0 = cpool.tile([64, QB], f16, tag="cs0")
                cs1 = cpool.tile([64, QB], f16, tag="cs1")
                sums = spool.tile([65, 2 * QB], f16, tag="sums")
                nc.vector.tensor_copy(cs0[:], ctxp0[0:64, :])
                nc.vector.tensor_copy(cs1[:], ctxp1[0:64, :])
                nc.vector.tensor_copy(sums[64:65, 0:QB], ctxp0[64:65, :])
                nc.vector.tensor_copy(sums[64:65, QB:2 * QB], ctxp1[64:65, :])
                rb0 = cp.tile([64, QB], f32, tag="ctx0")
                rb1 = cp.tile([64, QB], f32, tag="ctx1")
                mm(rb0[:], ones_t[64:65, :], sums[64:65, 0:QB],
                   start=True, stop=True)
                mm(rb1[:], ones_t[64:65, :], sums[64:65, QB:2 * QB],
                   start=True, stop=True)
                rec = rpool.tile([64, 2, QB], f32, tag="rec")
                nc.vector.reciprocal_approx_fast(rec[:, 0, :], rb0[:])
                nc.vector.reciprocal_approx_fast(rec[:, 1, :], rb1[:])
                nc.vector.tensor_mul(cs0[:], cs0[:], rec[:, 0, :])
                nc.vector.tensor_mul(cs1[:], cs1[:], rec[:, 1, :])

                # out-proj: out[m-block, :] = cs0.T@wo0 + cs1.T@wo1
                for m in range(QB // 128):
                    for nh in range(D // 512):
                        op = cp.tile([128, 512], f32, tag="ctx%d" % (m % 2))
                        mm(op[:], cs0[:, m * 128:(m + 1) * 128],
                           wo0_t[:, nh * 512:(nh + 1) * 512],
                           start=True, stop=False)
                        mm(op[:], cs1[:, m * 128:(m + 1) * 128],
                           wo1_t[:, nh * 512:(nh + 1) * 512],
                           start=False, stop=True)
                        ob = opool.tile([128, 512], f16, tag="ob")
                        nc.vector.tensor_copy(ob[:], op[:])
                        nc.sync.dma_start(
                            out_d[Q * QB + m * 128:Q * QB + (m + 1) * 128,
                                  nh * 512:(nh + 1) * 512],
                            ob[:])

    nc.compile()
    return nc


def _npdt16():
    if DT16 == 'bf16':
        import ml_dtypes
        return ml_dtypes.bfloat16
    return np.float16


def _shard_inputs(x, wq, bq, wk, bk, wv, bv, wo, bo, s):
    npdt16 = _npdt16()
    # [D, s] -> contiguous per-block layout [s//512, 128, D//128, 512], fp16
    xT2 = np.asarray(x, np.float32).reshape(s, D).T
    xT = np.ascontiguousarray(
        xT2.reshape(D // 128, 128, s // 512, 512).transpose(2, 1, 0, 3)
    ).astype(npdt16)

    def lhsT_layout(w, c):
        blk = np.asarray(w, np.float32)[:, c * 128:(c + 1) * 128]
        return np.ascontiguousarray(
            blk.reshape(DC, 128, 128).transpose(1, 0, 2).reshape(128, D)
        ).astype(npdt16)

    def wv_aug_layout(w, c):
        # [128, DC, 130]: per d-chunk, [h0 cols | 0 | h1 cols | 0]
        blk = np.asarray(w, np.float32)[:, c * 128:(c + 1) * 128]  # [D, 128]
        aug = np.zeros((DC, 128, 130), np.float32)
        aug[:, :, 0:64] = blk[:, 0:64].reshape(DC, 128, 64)
        aug[:, :, 65:129] = blk[:, 64:128].reshape(DC, 128, 64)
        return np.ascontiguousarray(aug.transpose(1, 0, 2)).astype(npdt16)

    def bvb_layout(bv, c):
        # [128, 2, 65]: v bias broadcast over k-rows + ones column
        bvc = np.asarray(bv, np.float32)[c * 128:(c + 1) * 128]
        t = np.empty((2, 65), np.float32)
        t[0, 0:64] = bvc[0:64]
        t[1, 0:64] = bvc[64:128]
        t[:, 64] = 1.0
        return np.ascontiguousarray(
            np.broadcast_to(t, (128, 2, 65))).astype(npdt16)

    in_maps = []
    for c in range(N_CORES):
        in_maps.append({
            "xT": xT,
            "wq": lhsT_layout(wq, c),
            "wk": lhsT_layout(wk, c),
            "wv": wv_aug_layout(wv, c),
            "bq": np.ascontiguousarray(
                np.asarray(bq, np.float32)[c * 128:(c + 1) * 128, None]),
            "bk": np.ascontiguousarray(
                np.asarray(bk, np.float32)[c * 128:(c + 1) * 128, None]),
            "bvb": bvb_layout(bv, c),
            "wo": np.ascontiguousarray(
                np.asarray(wo, np.float32)[c * 128:(c + 1) * 128, :]
            ).astype(npdt16),
        })
    return in_maps


def run(x, wq, bq, wk, bk, wv, bv, wo, bo, trace=False, s=S):
    global _LAST_EXEC_NS
    from concourse.bass_utils import run_bass_kernel_spmd

    if trace:
        _install_ntff_hook_shim()
    nc = _build(s)
    in_maps = _shard_inputs(x, wq, bq, wk, bk, wv, bv, wo, bo, s)
    res = run_bass_kernel_spmd(nc, in_maps, core_ids=list(range(N_CORES)),
                               trace=trace)
    _LAST_EXEC_NS = res.exec_time_ns
    out = res.results[0]["out"].astype(np.float64)
    for c in range(1, N_CORES):
        out += res.results[c]["out"]
    out += np.asarray(bo, np.float64)
    return out.astype(np.float32).reshape(1, s, D)


def kernel(x, wq, bq, wk, bk, wv, bv, wo, bo):
    trace = bool(os.environ.get("BASS_MHA_TRACE"))
    return run(x, wq, bq, wk, bk, wv, bv, wo, bo, trace=trace)


# revision 10
# speedup vs baseline: 1.2672x; 1.0378x over previous
"""Multi-head attention (B=1, S=4096, D=1024, H=16, Hd=64) on 8 Trainium2 cores.

Sharding: tensor-parallel over heads - 2 heads per core. Each core computes
q/k/v projections for its 2 heads (128 dims), flash-style attention without
max-subtraction (scores are ~N(0,1) after scaling so exp never overflows),
and a partial output projection with its 128 rows of wo. Host sums the 8
partial outputs and adds bo.

The exp stream on the scalar (ACT) engine is the roofline: 2 heads x 4096^2
= 33.5M exps per core at 1 elem/cycle/lane @1.2GHz ~= 252us including
per-instruction overhead. The kernel is a single flat software pipeline
built to keep ACT streaming:

  - scores are staged in PSUM groups of 3x[128,512], double buffered; the
    score matmuls for group j+2 are emitted while ctx matmuls consume group
    j, so the ACT engine always has the next group when it finishes one.
  - the score stream runs FLAT across q-block boundaries; per-block work
    (projections during q-block 0, the normalize/out-proj epilogue of block
    b during early groups of block b+1) is emitted as small filler pieces
    between groups so no contiguous PE blob ever starves ACT.
  - all matmul operands are fp16 (FWL halves weight loads, DMA halves);
    PSUM stays fp32. 16-bit gives no moving-stream speedup on this HW.
  - v is produced directly in attention layout [k-rows, head, 65] using the
    x^T chunk as stationary and an augmented wv (64 cols h0 | 0 | 64 cols
    h1 | 0) as moving; a host-prepared bias tile adds bv and the ones
    column (softmax denominator rides along as ctx row 64).
  - out-proj is a single K=128 matmul per output tile: the two heads'
    normalized ctx rows are stacked into one [128, 512] tile (cs01).
  - normalization: reciprocal of the two [1,512] denominator rows first,
    then a K=1 ones-matmul broadcasts 1/denom over partitions, then one
    tensor_mul on cs01.

Layouts on device (per core):
  xT   [8, 128, 512] fp16 per block: partitions = d-chunk dims
  qT/kT[128, S] fp16   partitions = head dims (h0: 0-63, h1: 64-127)
  v4   [128, 2, 65] fp16 per k-chunk: partitions = seq rows, col 64 = ones
  scores psum [128 (k rows), 3x512 (q)] fp32 -> exp on ACT -> ex fp16
  ctx^T psum [65, 512] fp32 per head, accumulated over 32 k-chunks
  out  [S, D] fp16 partials, summed + bo on host
"""

import os
import sys
import types

import numpy as np

S = 4096
D = 1024
H = 16
HD = 64
N_CORES = 8
HPC = H // N_CORES  # heads per core = 2
DC = D // 128       # d-chunks = 8
QB = 512            # q block

_LAST_EXEC_NS = None


def _install_ntff_hook_shim():
    if "antenv.axon_hooks" in sys.modules:
        return
    try:
        import antenv
        from trn_agent_boot.trn_boot import _ntff_profile_via_ctypes

        hook = _ntff_profile_via_ctypes("/opt/axon/libaxon_pjrt.so")
    except Exception:
        return
    mod = types.ModuleType("antenv.axon_hooks")
    _state = {"hook": hook}
    mod.get_axon_ntff_profile_hook = lambda: _state["hook"]
    mod.set_axon_ntff_profile_hook = lambda h: _state.update(hook=h)
    sys.modules["antenv.axon_hooks"] = mod
    antenv.axon_hooks = mod


def _build(s=S):
    import concourse.bass as bass
    import concourse.mybir as mybir
    import concourse.tile as tile
    from concourse import bacc

    f32 = mybir.dt.float32
    f32r = mybir.dt.float32r
    f16 = mybir.dt.float16
    Exp = mybir.ActivationFunctionType.Exp

    KC = s // 128     # k-chunks
    PB = 512          # projection block
    NP = s // PB      # projection / q blocks
    QB = 512
    GS = 3            # (kc, h) slices per exp staging group
    LOOK = 2          # score groups emitted ahead of ctx

    nc = bacc.Bacc("TRN2", target_bir_lowering=False, debug=False,
                   num_devices=N_CORES)

    xT_d = nc.declare_dram_parameter("xT", [NP, 128, DC, 512], f16,
                                     isOutput=False)
    wq_d = nc.declare_dram_parameter("wq", [128, D], f16, isOutput=False)
    wk_d = nc.declare_dram_parameter("wk", [128, D], f16, isOutput=False)
    wv_d = nc.declare_dram_parameter("wv", [128, DC, 130], f16, isOutput=False)
    bq_d = nc.declare_dram_parameter("bq", [128, 1], f32, isOutput=False)
    bk_d = nc.declare_dram_parameter("bk", [128, 1], f32, isOutput=False)
    bvb_d = nc.declare_dram_parameter("bvb", [128, 2, 65], f16, isOutput=False)
    wo_d = nc.declare_dram_parameter("wo", [128, D], f16, isOutput=False)
    out_d = nc.declare_dram_parameter("out", [s, D], f16, isOutput=True)

    with tile.TileContext(nc) as tc:
        import contextlib
        with contextlib.ExitStack() as ctx:
            wpool = ctx.enter_context(tc.tile_pool(name="w", bufs=1))
            xpool = ctx.enter_context(tc.tile_pool(name="x", bufs=NP))
            kpool = ctx.enter_context(tc.tile_pool(name="kt", bufs=1))
            qpool = ctx.enter_context(tc.tile_pool(name="qt", bufs=NP))
            vpool = ctx.enter_context(tc.tile_pool(name="v4", bufs=KC))
            epool = ctx.enter_context(tc.tile_pool(name="ex", bufs=6))
            cpool = ctx.enter_context(tc.tile_pool(name="ctxs", bufs=2))
            spool = ctx.enter_context(tc.tile_pool(name="sums", bufs=2))
            opool = ctx.enter_context(tc.tile_pool(name="outs", bufs=4))
            # PSUM: 2x3 (stage ring: scores + all proj/epilogue psum) +
            # 1 (ctx0) + 1 (ctx1) = 8 banks. The cp ring holds ONLY the ctx
            # accumulators so nothing long-lived ever blocks the stage ring.
            stg = ctx.enter_context(tc.tile_pool(name="stg", bufs=2, space="PSUM"))
            cp = ctx.enter_context(tc.tile_pool(name="cp", bufs=1, space="PSUM"))

            # ---- constants / weights ----
            wq_t = wpool.tile([128, D], f16, tag="wq")
            wk_t = wpool.tile([128, D], f16, tag="wk")
            wv_t = wpool.tile([128, DC, 130], f16, tag="wv")
            wo_t = wpool.tile([128, D], f16, tag="wo")
            bq_t = wpool.tile([128, 1], f32, tag="bq")
            bk_t = wpool.tile([128, 1], f32, tag="bk")
            bvb_t = wpool.tile([128, 2, 65], f16, tag="bvb")
            ones_f = wpool.tile([65, 64], f32, tag="ones_f")
            ones_t = wpool.tile([65, 64], f32r, tag="ones")

            nc.sync.dma_start(wq_t[:], wq_d[:])
            nc.sync.dma_start(wk_t[:], wk_d[:])
            nc.sync.dma_start(wv_t[:], wv_d[:])
            nc.sync.dma_start(wo_t[:], wo_d[:])
            nc.sync.dma_start(bq_t[:], bq_d[:])
            nc.sync.dma_start(bk_t[:], bk_d[:])
            nc.sync.dma_start(bvb_t[:], bvb_d[:])
            nc.vector.memset(ones_f[:], 1.0)
            nc.vector.tensor_copy(ones_t[:], ones_f[:])

            kT = kpool.tile([128, s], f16, tag="kT")
            q_tiles = [None] * NP
            v_tiles = [None] * KC

            def mm(out, lhsT, rhs, start, stop, tile_position=None):
                return nc.tensor.matmul(out, lhsT, rhs, start=start,
                                        stop=stop, tile_position=tile_position)

            # ---- x: all blocks DMA'd up front (free-running stream) ----
            x_tiles = []
            for b in range(NP):
                xb = xpool.tile([128, DC, PB], f16, tag="xb")
                nc.sync.dma_start(xb[:], xT_d[b])
                x_tiles.append(xb)

            # ---- projection emitters ----
            def emit_kq(b):
                xb = x_tiles[b]
                ps = stg.tile([128, PB], f32, tag="stage")
                for c in range(DC):
                    mm(ps[:], wk_t[:, c * 128:(c + 1) * 128], xb[:, c, :],
                       start=(c == 0), stop=(c == DC - 1))
                nc.vector.tensor_scalar_add(kT[:, b * PB:(b + 1) * PB],
                                            ps[:], bk_t[:])
                qb = qpool.tile([128, PB], f16, tag="qT")
                ps = stg.tile([128, PB], f32, tag="stage")
                for c in range(DC):
                    mm(ps[:], wq_t[:, c * 128:(c + 1) * 128], xb[:, c, :],
                       start=(c == 0), stop=(c == DC - 1))
                nc.vector.tensor_scalar_add(qb[:], ps[:], bq_t[:])
                q_tiles[b] = qb

            def emit_v(b, j):
                xb = x_tiles[b]
                kc = b * 4 + j
                vps = stg.tile([128, 130], f32, tag="stage")
                for c in range(DC):
                    mm(vps[:], xb[:, c, j * 128:(j + 1) * 128],
                       wv_t[:, c, :], start=(c == 0), stop=(c == DC - 1))
                v4 = vpool.tile([128, 2, 65], f16, tag="v4")
                nc.vector.tensor_add(
                    v4[:], vps[:].rearrange("p (h m) -> p h m", h=2),
                    bvb_t[:])
                v_tiles[kc] = v4

            # ---- attention stream plumbing ----
            slices = [(kc, h) for kc in range(KC) for h in range(2)]
            groups = [slices[i:i + GS] for i in range(0, len(slices), GS)]
            NG = len(groups)
            items = [(b, gi) for b in range(NP) for gi in range(NG)]

            def emit_scores_exp(b, gi):
                grp = groups[gi]
                ns = len(grp)
                qb = q_tiles[b]
                st = stg.tile([128, GS, QB], f32, tag="stage")
                ex = epool.tile([128, GS, QB], f16, tag="ex")
                for slot, (kc, h) in enumerate(grp):
                    mm(st[:, slot, :],
                       kT[h * 64:(h + 1) * 64, kc * 128:(kc + 1) * 128],
                       qb[h * 64:(h + 1) * 64, :],
                       start=True, stop=True)
                nc.scalar.activation(
                    ex[:, 0:ns, :], st[:, 0:ns, :], Exp,
                    bias=0.0, scale=float(1.0 / np.sqrt(HD)))
                return ex

            # normalize block b's ctx accumulators -> cs01 (frees cp ring)
            def emit_normalize(b, ctxp0, ctxp1):
                # h0 normalized in cs01[0:64]; h1 in its own base-0 tile
                # (DVE has no cross-lane path), then DMA'd into cs01[64:].
                cs01 = cpool.tile([128, QB], f16, tag="cs01")
                cs1t = cpool.tile([64, QB], f16, tag="cs1t")
                sums = spool.tile([65, 2 * QB], f32r, tag="sums")
                nc.vector.tensor_copy(cs01[0:64, :], ctxp0[0:64, :])
                nc.vector.tensor_copy(cs1t[:], ctxp1[0:64, :])
                nc.vector.tensor_copy(sums[64:65, 0:QB], ctxp0[64:65, :])
                nc.vector.tensor_copy(sums[64:65, QB:2 * QB],
                                      ctxp1[64:65, :])
                rb0 = stg.tile([64, QB], f32, tag="stage")
                mm(rb0[:], ones_t[64:65, :],
                   sums[64:65, 0:QB], start=True, stop=True)
                rb1 = stg.tile([64, QB], f32, tag="stage")
                mm(rb1[:], ones_t[64:65, :],
                   sums[64:65, QB:2 * QB], start=True, stop=True)
                rec = spool.tile([64, 2, QB], f32, tag="rec")
                nc.vector.reciprocal_approx_fast(rec[:, 0, :], rb0[:])
                nc.vector.reciprocal_approx_fast(rec[:, 1, :], rb1[:])
                nc.vector.tensor_mul(cs01[0:64, :], cs01[0:64, :],
                                     rec[:, 0, :])
                nc.vector.tensor_mul(cs1t[:], cs1t[:], rec[:, 1, :])
                nc.sync.dma_start(cs01[64:128, :], cs1t[:])
                return cs01

            # one out-proj piece: out[Q*QB + m*128 ... , nh*512 ...]
            def emit_out_piece(b, cs01, m, nh):
                op = stg.tile([128, 512], f32, tag="stage")
                mm(op[:], cs01[:, m * 128:(m + 1) * 128],
                   wo_t[:, nh * 512:(nh + 1) * 512], start=True, stop=True)
                ob = opool.tile([128, 512], f16, tag="ob")
                nc.vector.tensor_copy(ob[:], op[:])
                nc.sync.dma_start(
                    out_d[b * QB + m * 128:b * QB + (m + 1) * 128,
                          nh * 512:(nh + 1) * 512],
                    ob[:])

            # ---- phase A: blocks 0,1 projected up front ----
            for b in (0, 1):
                emit_kq(b)
                for j in range(4):
                    emit_v(b, j)

            # filler schedule: {global ctx iteration: [callable, ...]}
            fillers = {}

            def add_filler(i, fn):
                fillers.setdefault(i, []).append(fn)

            # projections of blocks 2..7 spread through q-block 0.
            # k-proj of block p must precede the score frontier needing
            # kc=4p (group ceil(8p/3), frontier = iteration - LOOK).
            for p in range(2, NP):
                base = 3 * (p - 2)  # p=2 -> 0 ... p=7 -> 15
                add_filler(base, (lambda pp: lambda: emit_kq(pp))(p))
                for j in range(4):
                    add_filler(base + 1 + (j // 2),
                               (lambda pp, jj: lambda: emit_v(pp, jj))(p, j))

            # ---- the flat stream ----
            j = 0
            ex_store = {}
            pend_out = []  # deferred out-proj pieces of the previous block
            ctxp0 = ctxp1 = None
            for i, (b, gi) in enumerate(items):
                if gi == 0:
                    ctxp0 = cp.tile([65, QB], f32, tag="ctx0")
                    ctxp1 = cp.tile([65, QB], f32, tag="ctx1")
                # keep the score/exp stream LOOK groups ahead
                while j < len(items) and j <= i + LOOK:
                    bj, gj = items[j]
                    ex_store[j] = emit_scores_exp(bj, gj)
                    j += 1
                # fillers for this iteration (projections during block 0)
                for fn in fillers.pop(i, ()):
                    fn()
                # deferred epilogue pieces of the previous q-block
                if pend_out and gi > 0:
                    pb, pcs, pm, pnh = pend_out.pop(0)
                    emit_out_piece(pb, pcs, pm, pnh)
                    if pend_out and gi % 2 == 0:
                        pb, pcs, pm, pnh = pend_out.pop(0)
                        emit_out_piece(pb, pcs, pm, pnh)
                # ctx accumulation for group gi
                ex = ex_store.pop(i)
                for slot, (kc, h) in enumerate(groups[gi]):
                    ctxp = ctxp0 if h == 0 else ctxp1
                    mm(ctxp[:], v_tiles[kc][:, h, :], ex[:, slot, :],
                       start=(kc == 0), stop=(kc == KC - 1))
                if gi == NG - 1:
                    # epilogue: normalize now (frees ctx ring for b+1);
                    # out-proj pieces trail into the next block's groups.
                    cs01 = emit_normalize(b, ctxp0, ctxp1)
                    pieces = [(b, cs01, m, nh)
                              for m in range(QB // 128)
                              for nh in range(D // 512)]
                    if b + 1 < NP:
                        pend_out.extend(pieces)
                    else:
                        for pb, pcs, pm, pnh in pieces:
                            emit_out_piece(pb, pcs, pm, pnh)
            # flush any stragglers
            for pb, pcs, pm, pnh in pend_out:
                emit_out_piece(pb, pcs, pm, pnh)

    nc.compile()
    return nc


def _shard_inputs(x, wq, bq, wk, bk, wv, bv, wo, bo, s):
    npdt16 = np.float16
    # [D, s] -> contiguous per-block layout [s//512, 128, D//128, 512]
    xT2 = np.asarray(x, np.float32).reshape(s, D).T
    xT = np.ascontiguousarray(
        xT2.reshape(D // 128, 128, s // 512, 512).transpose(2, 1, 0, 3)
    ).astype(npdt16)

    def lhsT_layout(w, c):
        blk = np.asarray(w, np.float32)[:, c * 128:(c + 1) * 128]
        return np.ascontiguousarray(
            blk.reshape(DC, 128, 128).transpose(1, 0, 2).reshape(128, D)
        ).astype(npdt16)

    def wv_aug_layout(w, c):
        # [128, DC, 130]: per d-chunk, [h0 cols | 0 | h1 cols | 0]
        blk = np.asarray(w, np.float32)[:, c * 128:(c + 1) * 128]  # [D, 128]
        aug = np.zeros((DC, 128, 130), np.float32)
        aug[:, :, 0:64] = blk[:, 0:64].reshape(DC, 128, 64)
        aug[:, :, 65:129] = blk[:, 64:128].reshape(DC, 128, 64)
        return np.ascontiguousarray(aug.transpose(1, 0, 2)).astype(npdt16)

    def bvb_layout(bv, c):
        # [128, 2, 65]: v bias broadcast over k-rows + ones column
        bvc = np.asarray(bv, np.float32)[c * 128:(c + 1) * 128]
        t = np.empty((2, 65), np.float32)
        t[0, 0:64] = bvc[0:64]
        t[1, 0:64] = bvc[64:128]
        t[:, 64] = 1.0
        return np.ascontiguousarray(
            np.broadcast_to(t, (128, 2, 65))).astype(npdt16)

    in_maps = []
    for c in range(N_CORES):
        in_maps.append({
            "xT": xT,
            "wq": lhsT_layout(wq, c),
            "wk": lhsT_layout(wk, c),
            "wv": wv_aug_layout(wv, c),
            "bq": np.ascontiguousarray(
                np.asarray(bq, np.float32)[c * 128:(c + 1) * 128, None]),
            "bk": np.ascontiguousarray(
                np.asarray(bk, np.float32)[c * 128:(c + 1) * 128, None]),
            "bvb": bvb_layout(bv, c),
            "wo": np.ascontiguousarray(
                np.asarray(wo, np.float32)[c * 128:(c + 1) * 128, :]
            ).astype(npdt16),
        })
    return in_maps


def run(x, wq, bq, wk, bk, wv, bv, wo, bo, trace=False, s=S):
    global _LAST_EXEC_NS
    from concourse.bass_utils import run_bass_kernel_spmd

    if trace:
        _install_ntff_hook_shim()
    nc = _build(s)
    in_maps = _shard_inputs(x, wq, bq, wk, bk, wv, bv, wo, bo, s)
    res = run_bass_kernel_spmd(nc, in_maps, core_ids=list(range(N_CORES)),
                               trace=trace)
    _LAST_EXEC_NS = res.exec_time_ns
    out = res.results[0]["out"].astype(np.float64)
    for c in range(1, N_CORES):
        out += res.results[c]["out"]
    out += np.asarray(bo, np.float64)
    return out.astype(np.float32).reshape(1, s, D)


def kernel(x, wq, bq, wk, bk, wv, bv, wo, bo):
    trace = bool(os.environ.get("BASS_MHA_TRACE"))
    return run(x, wq, bq, wk, bk, wv, bv, wo, bo, trace=trace)
